# revision 27
# baseline (speedup 1.0000x reference)
"""3x3 MedianBlur (zero-padded) for (8, 3, 1024, 1024) fp32 on 8 trn2 NeuronCores.

Strategy (~245us vs the 480us fp32 baseline):
  - Pure data parallel: batch element i -> core i.
  - bf16 compute: tolerance is 2e-2 rel; the median network is selection-
    exact, so bf16 costs only input rounding (~4e-3 rel, self-relative).
    Host converts fp32->bf16 before upload and bf16->fp32 after download,
    halving HBM traffic AND putting every DVE tensor_tensor in 2x_1p mode
    (0.5 cyc/elem instead of 1).
  - Flattened row-group layout: partition p holds image rows 8p-1..8p+8
    (8 output rows + 1 halo row each side) as 10 rows x 1026 cols (1 zero
    pad col each side) in the free dim. Both 3x3 window shifts are then
    free-dim offsets (+-1026 vertical, +-1 horizontal): one DMA load per
    channel instead of 3 row-shifted loads, no cross-partition traffic.
  - Exact separable median-of-9: 18 bf16 min/max tensor_tensor ops per
    channel, all on DVE -- the only 2-tensor min/max engine this walrus
    accepts (TensorTensor-on-Pool and InstPool both fail its ISA checks);
    exhaustive 0/1-domain search confirms the 6-op vertical sort and 4-op
    med3 stages are individually optimal. DVE ends ~97% busy.
  - Zero pads/halos via Pool-engine memsets; channel c+1 loads overlap
    channel c compute (bufs=2 on the x/out tiles); first/last channels are
    segmented by row-groups so the first DVE op starts right after the
    first sub-load and the final store tail is short.
"""
import sys

sys.path.insert(0, "/opt/trn_rl_repo")

import numpy as np

import concourse.bass as bass
import concourse.mybir as mybir
from concourse.bass_utils import run_bass_kernel_spmd
from concourse.tile import TileContext

C, H, W = 3, 1024, 1024
P = 128
RP = H // P          # rows per partition (8)
WP = W + 2           # padded row width (1026)
NR = RP + 2          # rows resident per partition incl. halo (10)
BF16 = mybir.dt.bfloat16
MIN = mybir.AluOpType.min
MAX = mybir.AluOpType.max


def _legalize_waits(nc):
    """Split sync_info.on_wait lists that exceed this walrus's per-instruction
    capacity (1; 2 for EventSemaphore) onto preceding same-engine NoOps."""
    for f in nc.m.functions:
        for bb in f.blocks:
            new_insts = []
            for ins in bb.instructions:
                si = ins.sync_info
                cap = 2 if ins.opcode == "EventSemaphore" else 1
                if si is not None and len(si.on_wait) > cap:
                    waits = list(si.on_wait)
                    extra, keep = waits[:-cap], waits[-cap:]
                    for w in extra:
                        nop = mybir.InstNoOp(
                            name=nc.get_next_instruction_name(),
                            ins=[],
                            outs=[],
                            engine=ins.engine,
                        )
                        nop.sync_info = mybir.SyncInfo(on_wait=[w], on_update=[])
                        new_insts.append(nop)
                    ins.sync_info = mybir.SyncInfo(
                        on_wait=keep, on_update=list(si.on_update)
                    )
                new_insts.append(ins)
            bb.instructions = new_insts


def build(segs_for=None, tail_cols=1):
    nc = bass.Bass()
    xd = nc.dram_tensor("x", [C, H, W], BF16, kind="ExternalInput")
    yd = nc.dram_tensor("y", [C, H, W], BF16, kind="ExternalOutput")
    tt = nc.vector.tensor_tensor

    if segs_for is None:
        # tuned in cost-model sim: tiny first segments start DVE as soon as
        # the first sub-load lands; tiny last segment shortens the store
        # tail. V and H stages are interleaved per segment — decoupling them
        # (all-V-then-all-H) measures ~1.3-3us slower in the timeline sim.
        segs_for = {
            0: [(0, 1), (1, 2), (2, RP)],
            1: [(0, RP)],
            2: [(0, 7), (7, RP)],
        }
    # normalize: each channel becomes an ordered schedule of
    # ('v'|'vh'|'h', k0, k1) entries; plain (k0,k1) means interleaved 'vh'.
    sched_for = {}
    for c, segs in segs_for.items():
        sched = []
        for e in segs:
            sched.append(e if len(e) == 3 else ("vh", e[0], e[1]))
        sched_for[c] = sched

    with TileContext(nc) as tc:
        with (
            tc.tile_pool(name="io", bufs=2) as iop,
            tc.tile_pool(name="work", bufs=1) as wp,
        ):
            for c in range(C):
                sched = sched_for[c]
                v_segs = [(k0, k1) for st, k0, k1 in sched if "v" in st]
                h_last = [(k0, k1) for st, k0, k1 in sched if "h" in st][-1]
                x = iop.tile([P, NR, WP], BF16, tag="x")
                nc.gpsimd.memset(x[:, :, 0:1], 0.0)
                nc.gpsimd.memset(x[:, :, WP - 1 : WP], 0.0)
                nc.gpsimd.memset(x[0:1, 0:1, :], 0.0)
                # compute engines must start at partition 0/32/64/96: zero
                # slot 9 for the whole last quadrant; the bottom-halo DMA
                # then overwrites partitions 96..126, leaving 127 zero.
                nc.gpsimd.memset(x[96:P, NR - 1 : NR, :], 0.0)
                # top halo first (gates the first V op), then main rows in
                # one chunk per segment, then bottom halo.
                nc.scalar.dma_start(
                    x[1:P, 0:1, 1 : W + 1],
                    xd[c : c + 1, RP - 1 : H - 1, :].rearrange(
                        "c (p k) w -> p (c k) w", k=RP
                    )[:, 0:1, :],
                )
                for k0, k1 in v_segs:
                    nc.sync.dma_start(
                        x[:, k0 + 1 : k1 + 1, 1 : W + 1],
                        xd[c : c + 1, :, :].rearrange(
                            "c (p k) w -> p (c k) w", k=RP
                        )[:, k0:k1, :],
                    )
                nc.gpsimd.dma_start(
                    x[0 : P - 1, NR - 1 : NR, 1 : W + 1],
                    xd[c : c + 1, RP:H, :].rearrange(
                        "c (p k) w -> p (c k) w", k=RP
                    )[:, 0:1, :],
                )

                u = wp.tile([P, RP, WP], BF16, tag="u")
                v = wp.tile([P, RP, WP], BF16, tag="v")
                w = wp.tile([P, RP, WP], BF16, tag="w")
                t1 = wp.tile([P, RP, WP - 1], BF16, tag="t1")
                t2 = wp.tile([P, RP, WP - 1], BF16, tag="t2")
                out = iop.tile([P, RP, W], BF16, tag="out")
                W1, W2 = WP - 1, WP - 2  # 1025, 1024

                for stage, k0, k1 in sched:
                    if "v" in stage:
                        # vertical stage: lo/me/hi of column triples for
                        # output row-slots [k0:k1) (x slots [k0:k1+2))
                        xm = x[:, k0 : k1, :]
                        x0 = x[:, k0 + 1 : k1 + 1, :]
                        xp = x[:, k0 + 2 : k1 + 2, :]
                        U = u[:, k0:k1]
                        V = v[:, k0:k1]
                        Wt = w[:, k0:k1]
                        tt(U[:, :, :], xm, x0, MIN)
                        tt(V[:, :, :], xm, x0, MAX)
                        tt(Wt[:, :, :], V[:, :, :], xp, MIN)
                        tt(Wt[:, :, :], U[:, :, :], Wt[:, :, :], MAX)  # M (me)
                        tt(U[:, :, :], U[:, :, :], xp, MIN)            # L (lo)
                        tt(V[:, :, :], V[:, :, :], xp, MAX)            # H (hi)
                    if "h" not in stage:
                        continue
                    # horizontal: med9 = med3(max3(L), med3(M), min3(H))
                    U = u[:, k0:k1]
                    V = v[:, k0:k1]
                    Wt = w[:, k0:k1]
                    T1 = t1[:, k0:k1]
                    T2 = t2[:, k0:k1]
                    tt(T1[:, :, :], U[:, :, 0:W1], U[:, :, 1:WP], MAX)            # a
                    tt(T1[:, :, 0:W2], T1[:, :, 0:W2], U[:, :, 2:WP], MAX)        # A
                    tt(U[:, :, 0:W1], V[:, :, 0:W1], V[:, :, 1:WP], MIN)          # cc
                    tt(U[:, :, 0:W2], U[:, :, 0:W2], V[:, :, 2:WP], MIN)          # Cc
                    tt(T2[:, :, :], Wt[:, :, 0:W1], Wt[:, :, 1:WP], MAX)          # q
                    tt(V[:, :, 0:W1], Wt[:, :, 0:W1], Wt[:, :, 1:WP], MIN)        # p
                    tt(T2[:, :, 0:W2], T2[:, :, 0:W2], Wt[:, :, 2:WP], MIN)       # b1
                    tt(T2[:, :, 0:W2], V[:, :, 0:W2], T2[:, :, 0:W2], MAX)        # B
                    tt(V[:, :, 0:W2], T1[:, :, 0:W2], T2[:, :, 0:W2], MIN)        # m1
                    tt(T1[:, :, 0:W2], T1[:, :, 0:W2], T2[:, :, 0:W2], MAX)       # m2
                    tt(T1[:, :, 0:W2], T1[:, :, 0:W2], U[:, :, 0:W2], MIN)        # m3
                    # final op + store; on the very last segment, split by
                    # columns so the last store is small (shorter tail) and
                    # issue it on the sync queue (smallest DGE delay)
                    last = c == C - 1 and (k0, k1) == h_last
                    col_splits = tail_cols if last else 1
                    cw = W // col_splits
                    tail_qs = [nc.sync, nc.gpsimd, nc.scalar]
                    for s in range(col_splits):
                        c0x, c1x = s * cw, (s + 1) * cw
                        tt(
                            out[:, k0:k1, c0x:c1x],
                            V[:, :, c0x:c1x],
                            T1[:, :, c0x:c1x],
                            MAX,
                        )  # med9
                        eng = tail_qs[s % len(tail_qs)] if last else nc.scalar
                        eng.dma_start(
                            yd[c : c + 1, :, :].rearrange(
                                "c (p k) w -> p (c k) w", k=RP
                            )[:, k0:k1, c0x:c1x],
                            out[:, k0:k1, c0x:c1x],
                        )

    _legalize_waits(nc)
    return nc


_NC = None


def kernel(input):
    import ml_dtypes

    global _NC
    if _NC is None:
        _NC = build()
    xb = np.asarray(input, dtype=np.float32).astype(ml_dtypes.bfloat16)
    in_maps = [{"x": np.ascontiguousarray(xb[i])} for i in range(xb.shape[0])]
    res = run_bass_kernel_spmd(_NC, in_maps, core_ids=list(range(len(in_maps))))
    return np.stack([r["y"] for r in res.results], axis=0).astype(np.float32)


# revision 28
# speedup vs baseline: 1.0002x; 1.0002x over previous
"""3x3 MedianBlur (zero-padded) for (8, 3, 1024, 1024) fp32 on 8 trn2 NeuronCores.

Strategy (~245us vs the 480us fp32 baseline):
  - Pure data parallel: batch element i -> core i.
  - bf16 compute: tolerance is 2e-2 rel; the median network is selection-
    exact, so bf16 costs only input rounding (~4e-3 rel, self-relative).
    Host converts fp32->bf16 before upload and bf16->fp32 after download,
    halving HBM traffic AND putting every DVE tensor_tensor in 2x_1p mode
    (0.5 cyc/elem instead of 1).
  - Flattened row-group layout: partition p holds image rows 8p-1..8p+8
    (8 output rows + 1 halo row each side) as 10 rows x 1026 cols (1 zero
    pad col each side) in the free dim. Both 3x3 window shifts are then
    free-dim offsets (+-1026 vertical, +-1 horizontal): one DMA load per
    channel instead of 3 row-shifted loads, no cross-partition traffic.
  - Exact separable median-of-9: 18 bf16 min/max tensor_tensor ops per
    channel, all on DVE -- the only 2-tensor min/max engine this walrus
    accepts (TensorTensor-on-Pool and InstPool both fail its ISA checks);
    exhaustive 0/1-domain search confirms the 6-op vertical sort and 4-op
    med3 stages are individually optimal. DVE ends ~97% busy.
  - Zero pads/halos via Pool-engine memsets; channel c+1 loads overlap
    channel c compute (bufs=2 on the x/out tiles); first/last channels are
    segmented by row-groups so the first DVE op starts right after the
    first sub-load and the final store tail is short.
"""
import sys

sys.path.insert(0, "/opt/trn_rl_repo")

import numpy as np

import concourse.bass as bass
import concourse.mybir as mybir
from concourse.bass_utils import run_bass_kernel_spmd
from concourse.tile import TileContext

C, H, W = 3, 1024, 1024
P = 128
RP = H // P          # rows per partition (8)
WP = W + 2           # padded row width (1026)
NR = RP + 2          # rows resident per partition incl. halo (10)
BF16 = mybir.dt.bfloat16
MIN = mybir.AluOpType.min
MAX = mybir.AluOpType.max


def _legalize_waits(nc):
    """Split sync_info.on_wait lists that exceed this walrus's per-instruction
    capacity (1; 2 for EventSemaphore) onto preceding same-engine NoOps."""
    for f in nc.m.functions:
        for bb in f.blocks:
            new_insts = []
            for ins in bb.instructions:
                si = ins.sync_info
                cap = 2 if ins.opcode == "EventSemaphore" else 1
                if si is not None and len(si.on_wait) > cap:
                    waits = list(si.on_wait)
                    extra, keep = waits[:-cap], waits[-cap:]
                    for w in extra:
                        nop = mybir.InstNoOp(
                            name=nc.get_next_instruction_name(),
                            ins=[],
                            outs=[],
                            engine=ins.engine,
                        )
                        nop.sync_info = mybir.SyncInfo(on_wait=[w], on_update=[])
                        new_insts.append(nop)
                    ins.sync_info = mybir.SyncInfo(
                        on_wait=keep, on_update=list(si.on_update)
                    )
                new_insts.append(ins)
            bb.instructions = new_insts


def build(segs_for=None, tail_cols=1):
    nc = bass.Bass()
    xd = nc.dram_tensor("x", [C, H, W], BF16, kind="ExternalInput")
    yd = nc.dram_tensor("y", [C, H, W], BF16, kind="ExternalOutput")
    tt = nc.vector.tensor_tensor

    if segs_for is None:
        # tuned in cost-model sim: tiny first segments start DVE as soon as
        # the first sub-load lands; tiny last segment shortens the store
        # tail. V and H stages are interleaved per segment — decoupling them
        # (all-V-then-all-H) measures ~1.3-3us slower in the timeline sim.
        segs_for = {
            0: [(0, 1), (1, 2), (2, RP)],
            1: [(0, RP)],
            2: [(0, 7), (7, RP)],
        }
    # normalize: each channel becomes an ordered schedule of
    # ('v'|'vh'|'h', k0, k1) entries; plain (k0,k1) means interleaved 'vh'.
    sched_for = {}
    for c, segs in segs_for.items():
        sched = []
        for e in segs:
            sched.append(e if len(e) == 3 else ("vh", e[0], e[1]))
        sched_for[c] = sched

    with TileContext(nc) as tc:
        with (
            tc.tile_pool(name="io", bufs=2) as iop,
            tc.tile_pool(name="work", bufs=1) as wp,
        ):
            for c in range(C):
                sched = sched_for[c]
                v_segs = [(k0, k1) for st, k0, k1 in sched if "v" in st]
                h_last = [(k0, k1) for st, k0, k1 in sched if "h" in st][-1]
                x = iop.tile([P, NR, WP], BF16, tag="x")
                nc.gpsimd.memset(x[:, :, 0:1], 0.0)
                nc.gpsimd.memset(x[:, :, WP - 1 : WP], 0.0)
                nc.gpsimd.memset(x[0:1, 0:1, :], 0.0)
                # compute engines must start at partition 0/32/64/96: zero
                # slot 9 for the whole last quadrant; the bottom-halo DMA
                # then overwrites partitions 96..126, leaving 127 zero.
                nc.gpsimd.memset(x[96:P, NR - 1 : NR, :], 0.0)
                # top halo first (gates the first V op), then main rows in
                # one chunk per segment, then bottom halo.
                nc.scalar.dma_start(
                    x[1:P, 0:1, 1 : W + 1],
                    xd[c : c + 1, RP - 1 : H - 1, :].rearrange(
                        "c (p k) w -> p (c k) w", k=RP
                    )[:, 0:1, :],
                )
                for k0, k1 in v_segs:
                    nc.sync.dma_start(
                        x[:, k0 + 1 : k1 + 1, 1 : W + 1],
                        xd[c : c + 1, :, :].rearrange(
                            "c (p k) w -> p (c k) w", k=RP
                        )[:, k0:k1, :],
                    )
                nc.gpsimd.dma_start(
                    x[0 : P - 1, NR - 1 : NR, 1 : W + 1],
                    xd[c : c + 1, RP:H, :].rearrange(
                        "c (p k) w -> p (c k) w", k=RP
                    )[:, 0:1, :],
                )

                u = wp.tile([P, RP, WP], BF16, tag="u")
                v = wp.tile([P, RP, WP], BF16, tag="v")
                w = wp.tile([P, RP, WP], BF16, tag="w")
                t1 = wp.tile([P, RP, WP - 1], BF16, tag="t1")
                t2 = wp.tile([P, RP, WP - 1], BF16, tag="t2")
                out = iop.tile([P, RP, W], BF16, tag="out")
                W1, W2 = WP - 1, WP - 2  # 1025, 1024

                for stage, k0, k1 in sched:
                    if "v" in stage:
                        # vertical stage: lo/me/hi of column triples for
                        # output row-slots [k0:k1) (x slots [k0:k1+2))
                        xm = x[:, k0 : k1, :]
                        x0 = x[:, k0 + 1 : k1 + 1, :]
                        xp = x[:, k0 + 2 : k1 + 2, :]
                        U = u[:, k0:k1]
                        V = v[:, k0:k1]
                        Wt = w[:, k0:k1]
                        tt(U[:, :, :], xm, x0, MIN)
                        tt(V[:, :, :], xm, x0, MAX)
                        tt(Wt[:, :, :], V[:, :, :], xp, MIN)
                        tt(Wt[:, :, :], U[:, :, :], Wt[:, :, :], MAX)  # M (me)
                        tt(U[:, :, :], U[:, :, :], xp, MIN)            # L (lo)
                        tt(V[:, :, :], V[:, :, :], xp, MAX)            # H (hi)
                    if "h" not in stage:
                        continue
                    # horizontal: med9 = med3(max3(L), med3(M), min3(H))
                    U = u[:, k0:k1]
                    V = v[:, k0:k1]
                    Wt = w[:, k0:k1]
                    T1 = t1[:, k0:k1]
                    T2 = t2[:, k0:k1]
                    tt(T1[:, :, 0:W2], U[:, :, 0:W2], U[:, :, 1 : WP - 1], MAX)   # a
                    tt(T1[:, :, 0:W2], T1[:, :, 0:W2], U[:, :, 2:WP], MAX)        # A
                    tt(U[:, :, 0:W2], V[:, :, 0:W2], V[:, :, 1 : WP - 1], MIN)    # cc
                    tt(U[:, :, 0:W2], U[:, :, 0:W2], V[:, :, 2:WP], MIN)          # Cc
                    tt(T2[:, :, 0:W2], Wt[:, :, 0:W2], Wt[:, :, 1 : WP - 1], MAX) # q
                    tt(V[:, :, 0:W2], Wt[:, :, 0:W2], Wt[:, :, 1 : WP - 1], MIN)  # p
                    tt(T2[:, :, 0:W2], T2[:, :, 0:W2], Wt[:, :, 2:WP], MIN)       # b1
                    tt(T2[:, :, 0:W2], V[:, :, 0:W2], T2[:, :, 0:W2], MAX)        # B
                    tt(V[:, :, 0:W2], T1[:, :, 0:W2], T2[:, :, 0:W2], MIN)        # m1
                    tt(T1[:, :, 0:W2], T1[:, :, 0:W2], T2[:, :, 0:W2], MAX)       # m2
                    tt(T1[:, :, 0:W2], T1[:, :, 0:W2], U[:, :, 0:W2], MIN)        # m3
                    # final op + store; on the very last segment, split by
                    # columns so the last store is small (shorter tail) and
                    # issue it on the sync queue (smallest DGE delay)
                    last = c == C - 1 and (k0, k1) == h_last
                    col_splits = tail_cols if last else 1
                    cw = W // col_splits
                    tail_qs = [nc.sync, nc.gpsimd, nc.scalar]
                    for s in range(col_splits):
                        c0x, c1x = s * cw, (s + 1) * cw
                        tt(
                            out[:, k0:k1, c0x:c1x],
                            V[:, :, c0x:c1x],
                            T1[:, :, c0x:c1x],
                            MAX,
                        )  # med9
                        eng = tail_qs[s % len(tail_qs)] if last else nc.scalar
                        eng.dma_start(
                            yd[c : c + 1, :, :].rearrange(
                                "c (p k) w -> p (c k) w", k=RP
                            )[:, k0:k1, c0x:c1x],
                            out[:, k0:k1, c0x:c1x],
                        )

    _legalize_waits(nc)
    return nc


_NC = None


def kernel(input):
    import ml_dtypes

    global _NC
    if _NC is None:
        _NC = build()
    xb = np.asarray(input, dtype=np.float32).astype(ml_dtypes.bfloat16)
    in_maps = [{"x": np.ascontiguousarray(xb[i])} for i in range(xb.shape[0])]
    res = run_bass_kernel_spmd(_NC, in_maps, core_ids=list(range(len(in_maps))))
    return np.stack([r["y"] for r in res.results], axis=0).astype(np.float32)
